# revision 7
# baseline (speedup 1.0000x reference)
"""PhaseLinear (Catmull-Rom spline-blended 4-way linear) on 8 trn2 cores.

out[b,o] = sum_c coeff(phase[b])[c] * (input[b] @ W[c].T + bias[c])[o]

Sharding: 8 cores = 4 batch groups x 2 out_feature halves.
Core r = bi*2 + oi handles input rows [bi*256,(bi+1)*256) and
out features [oi*256,(oi+1)*256).

Host-side layout prep (free): X and W are pre-transposed so the
contraction dim (in_features) lands on SBUF partitions for both matmul
operands, and control-point pairs are interleaved in the weight free
dim so one matmul streams N=512 covering two control points.

Per-core device kernel:
  - spline coeffs from phase as a cubic polynomial on DVE
    (t = (phase - sel*0.5pi)/(1.5pi); c_j = M0j + M1j t^2 + M2j t^3).
  - 4 fp32r matmul accumulation groups (m-tile x cp-pair), N=512.
  - bias blend via tiny K=4 matmul (coeff^T x biases).
  - final blend: chained scalar_tensor_tensor on DVE, coeff as
    per-partition scalar.
"""

import numpy as np

import concourse.bass as bass
import concourse.tile as tile
from concourse import bacc, mybir
from concourse.bass_utils import run_bass_kernel_spmd

N_CORES = 8
B, IN_F, OUT_F, NCP = 1024, 512, 512, 4
B_SH = B // 4        # 256 batch rows per core
O_SH = OUT_F // 2    # 256 out features per core
MT = B_SH // 128     # 2 m-tiles per core
KC = IN_F // 128     # 4 k-chunks
NG = NCP // 2        # 2 control-point pairs

# Catmull-Rom basis rows (only rows 0..2 used: tt = [1, t^2, t^3, 0])
CR = 0.5 * np.array(
    [[0.0, 2.0, 0.0, 0.0],
     [-1.0, 0.0, 1.0, 0.0],
     [2.0, -5.0, 4.0, -1.0],
     [-1.0, 3.0, -3.0, 1.0]], dtype=np.float64)

F32 = mybir.dt.float32
F32R = mybir.dt.float32r

_COMPILED = None


def _build():
    nc = bacc.Bacc("TRN2", target_bir_lowering=False, debug=False,
                   num_devices=N_CORES)

    # x^T: (IN_F, B_SH); w: (NG, IN_F, 2*O_SH) with cp pair interleaved in
    # the last dim; bias: (NCP, O_SH); ph: (128, MT); ident: (128, 128)
    xt_d = nc.dram_tensor("xt", [128, KC * B_SH], F32R,
                          kind="ExternalInput").ap()
    wt_d = nc.dram_tensor("wt", [NG, 128, KC * 2 * O_SH], F32R,
                          kind="ExternalInput").ap()
    b_d = nc.dram_tensor("b", [NCP, O_SH], F32R, kind="ExternalInput").ap()
    ph_d = nc.dram_tensor("ph", [128, MT], F32, kind="ExternalInput").ap()
    id_d = nc.dram_tensor("ident", [128, 128], F32, kind="ExternalInput").ap()
    y_d = nc.dram_tensor("y", [B_SH, O_SH], F32, kind="ExternalOutput").ap()

    k_t = 1.0 / (1.5 * np.pi)           # phase -> t scale
    thresh = 1.5 * np.pi                # segment select threshold

    with tile.TileContext(nc) as tc:
        with (
            tc.tile_pool(name="const", bufs=1) as cpool,
            tc.tile_pool(name="wts", bufs=NG) as wpool,
            tc.tile_pool(name="xt", bufs=1) as xtpool,
            tc.tile_pool(name="acc", bufs=4) as apool,
            tc.tile_pool(name="tps", bufs=2, space=bass.MemorySpace.PSUM) as tpsum,
            tc.tile_pool(name="y4", bufs=MT * NG,
                         space=bass.MemorySpace.PSUM) as ypsum,
            tc.tile_pool(name="bps", bufs=MT, space=bass.MemorySpace.PSUM) as bpsum,
        ):
            # ---- bulk loads (host-linearized: both sides row-contiguous;
            # weights split in k-halves so matmuls start earlier) ----
            xt_sb = xtpool.tile([128, KC * B_SH], F32R)
            nc.sync.dma_start(xt_sb[:], xt_d[:])
            wt_sb = []
            HW = KC * O_SH  # half the weight row (k-chunks 0..1)
            for g in range(NG):
                w = wpool.tile([128, KC * 2 * O_SH], F32R, tag="w",
                               name=f"w_{g}")
                wt_sb.append(w)
            for h in range(2):
                for g in range(NG):
                    nc.sync.dma_start(wt_sb[g][:, h * HW:(h + 1) * HW],
                                      wt_d[g][:, h * HW:(h + 1) * HW])

            # ---- small loads (on the ACT HWDGE queue) ----
            ph = cpool.tile([128, MT], F32)
            nc.scalar.dma_start(ph[:], ph_d[:])
            ident = cpool.tile([128, 128], F32)
            nc.scalar.dma_start(ident[:], id_d[:])
            bias_sb = cpool.tile([NCP, O_SH], F32R)
            nc.scalar.dma_start(bias_sb[:], b_d[:])

            # ---- PE warmup: keep HAM busy while weights stream in ----
            for wi in range(8):
                scratch = tpsum.tile([128, 128], F32, tag="tps",
                                     name=f"warm_{wi}")
                nc.tensor.matmul(scratch[:], ident[:], ident[:],
                                 start=True, stop=True)

            # ---- spline coefficients on DVE, all (128, MT) ----
            m = cpool.tile([128, MT], F32)
            nc.vector.tensor_scalar(m[:], ph[:], float(thresh), None,
                                    mybir.AluOpType.is_lt)
            t1 = cpool.tile([128, MT], F32)
            nc.vector.tensor_scalar(t1[:], ph[:], float(k_t), 1.0 / 3.0,
                                    mybir.AluOpType.mult,
                                    mybir.AluOpType.subtract)
            t = cpool.tile([128, MT], F32)
            nc.vector.scalar_tensor_tensor(t[:], m[:], 1.0 / 3.0, t1[:],
                                           mybir.AluOpType.mult,
                                           mybir.AluOpType.add)
            t2 = cpool.tile([128, MT], F32)
            nc.vector.tensor_mul(t2[:], t[:], t[:])
            t3 = cpool.tile([128, MT], F32)
            nc.vector.tensor_mul(t3[:], t2[:], t[:])
            # C[:, j*NCP + cp] = coeff for (m-tile j, control point cp)
            C = cpool.tile([128, MT * NCP], F32)
            for cp in range(NCP):
                u = cpool.tile([128, MT], F32, tag="u")
                nc.vector.tensor_scalar(u[:], t2[:], float(CR[1, cp]),
                                        float(CR[0, cp]),
                                        mybir.AluOpType.mult,
                                        mybir.AluOpType.add)
                dst = C[:, cp::NCP]  # columns cp, NCP+cp
                nc.vector.scalar_tensor_tensor(dst, t3[:], float(CR[2, cp]),
                                               u[:], mybir.AluOpType.mult,
                                               mybir.AluOpType.add)

            # ---- coeff^T (per m-tile) for the bias matmul ----
            ct_sb = []
            for j in range(MT):
                ct_ps = tpsum.tile([NCP, 128], F32, tag="tps",
                                   name=f"ctps_{j}")
                nc.tensor.transpose(ct_ps[:], C[:, j * NCP:(j + 1) * NCP],
                                    ident[:])
                ct = cpool.tile([NCP, 128], F32R, tag="ct", name=f"ct_{j}")
                nc.vector.tensor_scalar(ct[:], ct_ps[:], 1.0, None,
                                        mybir.AluOpType.mult)
                ct_sb.append(ct)

            # ---- matmuls + blend per m-tile ----
            for j in range(MT):
                bias_ps = bpsum.tile([128, O_SH], F32, tag="bps",
                                     name=f"bps_{j}")
                nc.tensor.matmul(bias_ps[:], ct_sb[j][:], bias_sb[:],
                                 start=True, stop=True)
                bias_sbuf = apool.tile([128, O_SH], F32, tag="biascp",
                                       name=f"biascp_{j}")
                nc.vector.tensor_scalar(bias_sbuf[:], bias_ps[:], 1.0, None,
                                        mybir.AluOpType.mult)
                y4 = [ypsum.tile([128, 2 * O_SH], F32, tag="y4",
                                 name=f"y4_{j}_{g}") for g in range(NG)]
                for g in range(NG):
                    for k in range(KC):
                        lhs = xt_sb[:, k * B_SH + j * 128:
                                    k * B_SH + (j + 1) * 128]
                        rhs = wt_sb[g][:, k * 2 * O_SH:(k + 1) * 2 * O_SH]
                        nc.tensor.matmul(y4[g][:], lhs, rhs,
                                         start=(k == 0), stop=(k == KC - 1))
                # blend: acc = bias + sum_cp c_cp * y4_cp
                prev = bias_sbuf
                for cp in range(NCP):
                    nxt = apool.tile([128, O_SH], F32, tag="acc",
                                     name=f"acc_{j}_{cp}")
                    src = y4[cp // 2][:, (cp % 2) * O_SH:(cp % 2 + 1) * O_SH]
                    col = j * NCP + cp
                    nc.vector.scalar_tensor_tensor(
                        nxt[:], src, C[:, col:col + 1], prev[:],
                        mybir.AluOpType.mult, mybir.AluOpType.add)
                    prev = nxt
                nc.sync.dma_start(
                    y_d[j * 128:(j + 1) * 128, :], prev[:])

    nc.compile()
    return nc


def _get_compiled():
    global _COMPILED
    if _COMPILED is None:
        _COMPILED = _build()
    return _COMPILED


def _shard_inputs(inputs):
    inp = np.ascontiguousarray(inputs["input"], dtype=np.float32)
    phase = np.ascontiguousarray(inputs["phase"], dtype=np.float32)
    weights = np.ascontiguousarray(inputs["weights"], dtype=np.float32)
    biases = np.ascontiguousarray(inputs["biases"], dtype=np.float32)

    # (NG, IN_F, 2, O_SH_full...) -> per pair g, w_pair[g][i, h*OUT_F..] with
    # the two control points of the pair side by side in the last dim
    w_t = weights.transpose(0, 2, 1)  # (NCP, IN_F, OUT_F)
    ident = np.eye(128, dtype=np.float32)

    in_maps = []
    for r in range(N_CORES):
        bi, oi = r // 2, r % 2
        osl = slice(oi * O_SH, (oi + 1) * O_SH)
        x_sh = inp[bi * B_SH:(bi + 1) * B_SH]          # (B_SH, IN_F)
        ph = phase[bi * B_SH:(bi + 1) * B_SH]
        # wt_lin[g][p, k*2*O_SH + h*O_SH + o] = W[2g+h, oi*O_SH+o, k*128+p]
        wt = np.empty((NG, 128, KC, 2, O_SH), dtype=np.float32)
        for g in range(NG):
            for h in range(2):
                # (IN_F, O_SH) -> (KC, 128, O_SH) -> (128, KC, O_SH)
                wt[g, :, :, h, :] = w_t[2 * g + h, :, osl].reshape(
                    KC, 128, O_SH).transpose(1, 0, 2)
        wt = np.ascontiguousarray(wt.reshape(NG, 128, KC * 2 * O_SH))
        # xt_lin[p, k*B_SH + b] = x_sh[b, k*128+p]
        xt = np.ascontiguousarray(
            x_sh.T.reshape(KC, 128, B_SH).transpose(1, 0, 2).reshape(
                128, KC * B_SH))
        in_maps.append({
            "xt": xt,
            "wt": wt,
            "b": np.ascontiguousarray(biases[:, osl]),
            "ph": np.ascontiguousarray(ph.reshape(MT, 128).T),
            "ident": ident,
        })
    return in_maps


def _run(inputs, trace=False, tmpdir=None, trace_cores=None):
    in_maps = _shard_inputs(inputs)
    nc = _get_compiled()
    res = run_bass_kernel_spmd(nc, in_maps, list(range(N_CORES)),
                               trace=trace, tmpdir=tmpdir,
                               trace_cores=trace_cores)
    out = np.empty((B, OUT_F), dtype=np.float32)
    for r in range(N_CORES):
        bi, oi = r // 2, r % 2
        out[bi * B_SH:(bi + 1) * B_SH, oi * O_SH:(oi + 1) * O_SH] = \
            res.results[r]["y"]
    return out, res


def kernel(**inputs):
    out, _ = _run(inputs)
    return out


# revision 8
# speedup vs baseline: 1.1257x; 1.1257x over previous
"""PhaseLinear (Catmull-Rom spline-blended 4-way linear) on 8 trn2 cores.

out[b,o] = sum_c coeff(phase[b])[c] * (input[b] @ W[c].T + bias[c])[o]

Sharding: 8 cores = 4 batch groups x 2 out_feature halves.
Core r = bi*2 + oi handles input rows [bi*256,(bi+1)*256) and
out features [oi*256,(oi+1)*256).

Host-side layout prep (free): X and W are pre-transposed/linearized so
the contraction dim lands on SBUF partitions for both matmul operands
and every DMA is row-contiguous on both sides; control-point pairs are
interleaved in the weight free dim so one matmul streams N=512 covering
two control points.

Device kernel: PE warmup matmuls (HAM unthrottle) on memset data while
weights stream; spline coeffs as a cubic polynomial on DVE; fp32r
matmul accumulation groups (m-tile x cp-pair) emitted k-major so the PE
chases the k-quartered weight DMA stream; bias blend via tiny K=4
matmul; final blend on DVE with per-partition coeff scalars.
"""

import numpy as np

import concourse.bass as bass
import concourse.tile as tile
from concourse import bacc, mybir
from concourse.bass_utils import run_bass_kernel_spmd

N_CORES = 8
B, IN_F, OUT_F, NCP = 1024, 512, 512, 4
B_SH = B // 4        # 256 batch rows per core
O_SH = OUT_F // 2    # 256 out features per core
MT = B_SH // 128     # 2 m-tiles per core
KC = IN_F // 128     # 4 k-chunks
NG = NCP // 2        # 2 control-point pairs
WROW = KC * 2 * O_SH  # weight row length per pair (2048)

# Catmull-Rom basis rows (only rows 0..2 used: tt = [1, t^2, t^3, 0])
CR = 0.5 * np.array(
    [[0.0, 2.0, 0.0, 0.0],
     [-1.0, 0.0, 1.0, 0.0],
     [2.0, -5.0, 4.0, -1.0],
     [-1.0, 3.0, -3.0, 1.0]], dtype=np.float64)

F32 = mybir.dt.float32
F32R = mybir.dt.float32r

_COMPILED = None


def _build():
    nc = bacc.Bacc("TRN2", target_bir_lowering=False, debug=False,
                   num_devices=N_CORES)

    xt_d = nc.dram_tensor("xt", [128, KC * B_SH], F32R,
                          kind="ExternalInput").ap()
    wt_d = nc.dram_tensor("wt", [NG, 128, WROW], F32R,
                          kind="ExternalInput").ap()
    b_d = nc.dram_tensor("b", [NCP, O_SH], F32R, kind="ExternalInput").ap()
    # const: cols 0-127 identity, cols 128..128+MT-1 phase tiles
    c_d = nc.dram_tensor("cst", [128, 128 + MT], F32,
                         kind="ExternalInput").ap()
    y_d = nc.dram_tensor("y", [B_SH, O_SH], F32, kind="ExternalOutput").ap()

    k_t = 1.0 / (1.5 * np.pi)           # phase -> t scale
    thresh = 1.5 * np.pi                # segment select threshold

    with tile.TileContext(nc) as tc:
        with (
            tc.tile_pool(name="const", bufs=1) as cpool,
            tc.tile_pool(name="wts", bufs=NG) as wpool,
            tc.tile_pool(name="xt", bufs=1) as xtpool,
            tc.tile_pool(name="acc", bufs=4) as apool,
            tc.tile_pool(name="tps", bufs=2, space=bass.MemorySpace.PSUM) as tpsum,
            tc.tile_pool(name="y4", bufs=MT * NG,
                         space=bass.MemorySpace.PSUM) as ypsum,
            tc.tile_pool(name="bps", bufs=MT, space=bass.MemorySpace.PSUM) as bpsum,
        ):
            # ---- bulk loads on the SP HWDGE ring (row-contiguous both
            # sides; weights k-quartered and pair-interleaved so the PE can
            # chase the stream) ----
            QW = WROW // KC  # one k-chunk of a pair row (512)
            xt_sb = xtpool.tile([128, KC * B_SH], F32R)
            nc.sync.dma_start(xt_sb[:], xt_d[:])
            wt_sb = [wpool.tile([128, WROW], F32R, tag="w", name=f"w_{g}")
                     for g in range(NG)]
            for k in range(KC):
                for g in range(NG):
                    nc.sync.dma_start(wt_sb[g][:, k * QW:(k + 1) * QW],
                                      wt_d[g][:, k * QW:(k + 1) * QW])

            # ---- small loads on the ACT HWDGE ring ----
            cst = cpool.tile([128, 128 + MT], F32)
            nc.scalar.dma_start(cst[:], c_d[:])
            ident = cst[:, 0:128]
            ph = cst[:, 128:128 + MT]
            bias_sb = cpool.tile([NCP, O_SH], F32R)
            nc.scalar.dma_start(bias_sb[:], b_d[:])

            # ---- PE warmup on memset data (no DMA dependency) ----
            warm_in = cpool.tile([128, 128], F32)
            nc.gpsimd.memset(warm_in[:], 0.0)
            for wi in range(4):
                scratch = tpsum.tile([128, 128], F32, tag="tps",
                                     name=f"warm_{wi}")
                nc.tensor.matmul(scratch[:], warm_in[:], warm_in[:],
                                 start=True, stop=True)

            # ---- spline coefficients on DVE, all (128, MT) ----
            m = cpool.tile([128, MT], F32)
            nc.vector.tensor_scalar(m[:], ph, float(thresh), None,
                                    mybir.AluOpType.is_lt)
            t1 = cpool.tile([128, MT], F32)
            nc.vector.tensor_scalar(t1[:], ph, float(k_t), 1.0 / 3.0,
                                    mybir.AluOpType.mult,
                                    mybir.AluOpType.subtract)
            t = cpool.tile([128, MT], F32)
            nc.vector.scalar_tensor_tensor(t[:], m[:], 1.0 / 3.0, t1[:],
                                           mybir.AluOpType.mult,
                                           mybir.AluOpType.add)
            t2 = cpool.tile([128, MT], F32)
            nc.vector.tensor_mul(t2[:], t[:], t[:])
            t3 = cpool.tile([128, MT], F32)
            nc.vector.tensor_mul(t3[:], t2[:], t[:])
            # C[:, j*NCP + cp] = coeff for (m-tile j, control point cp)
            C = cpool.tile([128, MT * NCP], F32)
            for cp in range(NCP):
                u = cpool.tile([128, MT], F32, tag="u")
                nc.vector.tensor_scalar(u[:], t2[:], float(CR[1, cp]),
                                        float(CR[0, cp]),
                                        mybir.AluOpType.mult,
                                        mybir.AluOpType.add)
                dst = C[:, cp::NCP]  # columns cp, NCP+cp
                nc.vector.scalar_tensor_tensor(dst, t3[:], float(CR[2, cp]),
                                               u[:], mybir.AluOpType.mult,
                                               mybir.AluOpType.add)

            # ---- coeff^T + bias blend matmul (small, before main mms) ----
            ct_sb, bias_sbuf = [], []
            for j in range(MT):
                ct_ps = tpsum.tile([NCP, 128], F32, tag="tps",
                                   name=f"ctps_{j}")
                nc.tensor.transpose(ct_ps[:], C[:, j * NCP:(j + 1) * NCP],
                                    ident)
                ct = cpool.tile([NCP, 128], F32R, tag="ct", name=f"ct_{j}")
                nc.scalar.copy(ct[:], ct_ps[:])
                ct_sb.append(ct)
            for j in range(MT):
                bias_ps = bpsum.tile([128, O_SH], F32, tag="bps",
                                     name=f"bps_{j}")
                nc.tensor.matmul(bias_ps[:], ct_sb[j][:], bias_sb[:],
                                 start=True, stop=True)
                bs = apool.tile([128, O_SH], F32, tag="biascp",
                                name=f"biascp_{j}")
                nc.scalar.copy(bs[:], bias_ps[:])
                bias_sbuf.append(bs)

            # ---- main matmuls, k-major so PE chases the weight stream ----
            y4 = [[ypsum.tile([128, 2 * O_SH], F32, tag="y4",
                              name=f"y4_{j}_{g}") for g in range(NG)]
                  for j in range(MT)]
            for k in range(KC):
                for g in range(NG):
                    for j in range(MT):
                        lhs = xt_sb[:, k * B_SH + j * 128:
                                    k * B_SH + (j + 1) * 128]
                        rhs = wt_sb[g][:, k * QW:(k + 1) * QW]
                        nc.tensor.matmul(y4[j][g][:], lhs, rhs,
                                         start=(k == 0), stop=(k == KC - 1))

            # ---- blend + store ----
            for j in range(MT):
                prev = bias_sbuf[j]
                for cp in range(NCP):
                    nxt = apool.tile([128, O_SH], F32, tag="acc",
                                     name=f"acc_{j}_{cp}")
                    src = y4[j][cp // 2][:, (cp % 2) * O_SH:
                                         (cp % 2 + 1) * O_SH]
                    col = j * NCP + cp
                    nc.vector.scalar_tensor_tensor(
                        nxt[:], src, C[:, col:col + 1], prev[:],
                        mybir.AluOpType.mult, mybir.AluOpType.add)
                    prev = nxt
                nc.sync.dma_start(y_d[j * 128:(j + 1) * 128, :], prev[:])

    nc.compile()
    return nc


def _get_compiled():
    global _COMPILED
    if _COMPILED is None:
        _COMPILED = _build()
    return _COMPILED


def _shard_inputs(inputs):
    inp = np.ascontiguousarray(inputs["input"], dtype=np.float32)
    phase = np.ascontiguousarray(inputs["phase"], dtype=np.float32)
    weights = np.ascontiguousarray(inputs["weights"], dtype=np.float32)
    biases = np.ascontiguousarray(inputs["biases"], dtype=np.float32)

    w_t = weights.transpose(0, 2, 1)  # (NCP, IN_F, OUT_F)
    cst_base = np.empty((128, 128 + MT), dtype=np.float32)
    cst_base[:, 0:128] = np.eye(128, dtype=np.float32)

    in_maps = []
    for r in range(N_CORES):
        bi, oi = r // 2, r % 2
        osl = slice(oi * O_SH, (oi + 1) * O_SH)
        x_sh = inp[bi * B_SH:(bi + 1) * B_SH]          # (B_SH, IN_F)
        ph = phase[bi * B_SH:(bi + 1) * B_SH]
        # wt_lin[g][p, k*2*O_SH + h*O_SH + o] = W[2g+h, oi*O_SH+o, k*128+p]
        wt = np.empty((NG, 128, KC, 2, O_SH), dtype=np.float32)
        for g in range(NG):
            for h in range(2):
                wt[g, :, :, h, :] = w_t[2 * g + h, :, osl].reshape(
                    KC, 128, O_SH).transpose(1, 0, 2)
        wt = np.ascontiguousarray(wt.reshape(NG, 128, WROW))
        # xt_lin[p, k*B_SH + b] = x_sh[b, k*128+p]
        xt = np.ascontiguousarray(
            x_sh.T.reshape(KC, 128, B_SH).transpose(1, 0, 2).reshape(
                128, KC * B_SH))
        cst = cst_base.copy()
        cst[:, 128:128 + MT] = ph.reshape(MT, 128).T
        in_maps.append({
            "xt": xt,
            "wt": wt,
            "b": np.ascontiguousarray(biases[:, osl]),
            "cst": cst,
        })
    return in_maps


def _run(inputs, trace=False, tmpdir=None, trace_cores=None):
    in_maps = _shard_inputs(inputs)
    nc = _get_compiled()
    res = run_bass_kernel_spmd(nc, in_maps, list(range(N_CORES)),
                               trace=trace, tmpdir=tmpdir,
                               trace_cores=trace_cores)
    out = np.empty((B, OUT_F), dtype=np.float32)
    for r in range(N_CORES):
        bi, oi = r // 2, r % 2
        out[bi * B_SH:(bi + 1) * B_SH, oi * O_SH:(oi + 1) * O_SH] = \
            res.results[r]["y"]
    return out, res


def kernel(**inputs):
    out, _ = _run(inputs)
    return out
